# revision 26
# baseline (speedup 1.0000x reference)
"""CMRGCN Trainium2 kernel (v2, fp16 matmul path).

Sharding: data-parallel over batch B=8 across the 8 NeuronCores (core b gets
batch b). Adjacency / neighbor weights / fused relation weights replicated.

v2 changes over the f32r baseline (361 us):
  - all matmul operands fp16 (PE row rate identical, but halves SBUF/DMA and
    enables DVE 2x modes on fp16 elementwise ops)
  - elementwise spread over ACT + DVE + Pool (gpsimd) instead of ACT+DVE only;
    ReLU runs on DVE as (psum + bias) max zeros via scalar_tensor_tensor
  - gather phase: psum rotation over two pools, fp16 staging tile double-
    buffered, one big DMA per (graph, channel-block, mix) instead of per-t
    copies racing a single-buffered staging tile (the old bottleneck: PE
    stalled ~100us waiting on staging WAR + 1.5MB DMAs)
  - output layout [C, T, 512] fp16 on device; host slices/transposes back to
    [C, N, T] f32

Per-core layout (N=500 padded to NP=512):
  g   [4 x (128, cb=3, T, MIX=2, DIM)] fp16, node-major. cb 0=x, 1=h1, 2=h2.
  Adjacency matmul: lhsT = g-slice [n, (i,d)], rhs = adj [n, m] -> P^T psum
  Weight matmul:    lhsT = fused-W block [ (j,d), (i,d') ], rhs = P^T
  d-path folded as +/-W pairs (relation weights for i!=j), i==j tanh(bias)
  terms folded into a per-layer constant added at merge time.
  h [128=(i,d'), T, m] fp16 accumulated via act tmps + 5-op merge tree,
  PE-transposed back into g c-block l+1.
  Neighbor gather = dense matmul with host-densified W_g [m, n] per graph.
"""

import os
import numpy as np

B, T, N, DIM = 8, 12, 500, 64
N_MIX, N_LAYERS, N_HEADS, N_REL, NG, K = 2, 2, 4, 8, 2, 20
NP = 512          # padded node count
NF = 500          # moving-free trim: only the first N output nodes are real
KT = NP // 128    # node k-tiles
C = DIM * (N_LAYERS + 1)   # 192 channels per mix in g
NCORES = 8

_BUILT = {}


def _rel(tg, i, j):
    return (tg * N_MIX + i) * N_MIX + j


def _build():
    if "nc" in _BUILT:
        return _BUILT["nc"]

    from contextlib import ExitStack
    import concourse.bass as bass
    import concourse.tile as tile
    import concourse.mybir as mybir
    from concourse import bacc
    from concourse.masks import make_identity

    f32 = mybir.dt.float32
    f16 = mybir.dt.float16
    AF = mybir.ActivationFunctionType
    ALU = mybir.AluOpType

    nc = bacc.Bacc("TRN2", target_bir_lowering=False, debug=False)

    xn_d = nc.dram_tensor("xn", [N_MIX, NP, T, DIM], f16, kind="ExternalInput").ap()
    adj_d = nc.dram_tensor("adj", [NG, NP, NP], f16, kind="ExternalInput").ap()
    wg_d = nc.dram_tensor("wg", [NG, NP, NP], f16, kind="ExternalInput").ap()
    # W blocks [128=(j,d), 128=(i,d')], K=128 with zero halves so every matmul
    # operand sits at base partition 0 (base-64 operands crash HW)
    wmm_d = nc.dram_tensor("wmm", [128, 12 * 128], f16, kind="ExternalInput").ap()
    bias_d = nc.dram_tensor("bias", [128, 16], f32, kind="ExternalInput").ap()
    out_d = [
        nc.dram_tensor(f"out{i}", [NG * C, T, NP], f16, kind="ExternalOutput").ap()
        for i in range(N_MIX)
    ]

    with tile.TileContext(nc) as tc, ExitStack() as ctx:
        wpool = ctx.enter_context(tc.tile_pool(name="wpool", bufs=1))
        gpool = ctx.enter_context(tc.tile_pool(name="gpool", bufs=1))
        gmpool = ctx.enter_context(tc.tile_pool(name="gmpool", bufs=1))
        ptpool = ctx.enter_context(tc.tile_pool(name="ptpool", bufs=2))
        hpool = ctx.enter_context(tc.tile_pool(name="hpool", bufs=2))
        apool = ctx.enter_context(tc.tile_pool(name="apool", bufs=12))
        mpool = ctx.enter_context(tc.tile_pool(name="mpool", bufs=8))
        outpool = ctx.enter_context(tc.tile_pool(name="outpool", bufs=2))
        psA = ctx.enter_context(tc.tile_pool(name="psA", bufs=2, space="PSUM"))
        psW = ctx.enter_context(tc.tile_pool(name="psW", bufs=4, space="PSUM"))
        psT = ctx.enter_context(tc.tile_pool(name="psT", bufs=2, space="PSUM"))

        # --- constants / weights ---
        wmm_sb = wpool.tile([128, 12 * 128], f16, name="wmm_sb")
        bias_sb = wpool.tile([128, 16], f32, name="bias_sb")
        ident = wpool.tile([128, 128], f16, name="ident")
        make_identity(nc, ident[:])
        zeros = wpool.tile([128, 2, NP], f16, name="zeros")
        nc.gpsimd.memset(zeros[:], 0.0)

        def wmm_blk(idx):
            return wmm_sb[:, idx * 128:(idx + 1) * 128]

        def c_wblk(l, tg, j):
            return wmm_blk((l * NG + tg) * 2 + j)

        def d_wblk(l, tg):
            return wmm_blk(8 + l * NG + tg)

        def c_bias(l, tg, j):
            col = (l * NG + tg) * 2 + j
            return bias_sb[:, col:col + 1]

        def d_bias(l, tg):
            return bias_sb[:, 8 + l * NG + tg: 8 + l * NG + tg + 1]

        def hconst(l):
            return bias_sb[:, 12 + l:12 + l + 1]

        # --- load x into g c-block 0, adjacency + gather-weight tiles ---
        # dispatch order tuned for startup: adjacency tg0 (sync) and xn
        # (split queues) first so the first A-phase chain starts ASAP; wg
        # last (first needed ~35us in by the interleaved gather rounds)
        g = []
        for mt in range(KT):
            gt = gpool.tile([128, 3, T, N_MIX, DIM], f16, name=f"g{mt}", tag=f"g{mt}")
            g.append(gt)

        # split the 12 startup-critical transfers (adj graph 0 + x) 6/6
        # across the two HWDGE queues so the first A chain starts earliest
        adj_sb, wg_sb = {}, {}
        for kt in range(KT):
            a = gmpool.tile([128, NP], f16, name=f"adj0{kt}", tag=f"gm0{kt}")
            eng = nc.sync if kt < 2 else nc.scalar
            eng.dma_start(out=a[:], in_=adj_d[0, kt * 128:(kt + 1) * 128, :])
            adj_sb[(0, kt)] = a
        for mt in range(KT):
            for i in range(N_MIX):
                eng = nc.sync if (mt + i) % 2 == 0 else nc.scalar
                eng.dma_start(
                    out=g[mt][:, 0, :, i, :],
                    in_=xn_d[i, mt * 128:(mt + 1) * 128, :, :],
                )
        for kt in range(KT):
            a = gmpool.tile([128, NP], f16, name=f"adj1{kt}", tag=f"gm1{kt}")
            eng = nc.scalar if kt < 2 else nc.sync
            eng.dma_start(out=a[:], in_=adj_d[1, kt * 128:(kt + 1) * 128, :])
            adj_sb[(1, kt)] = a
        for tg in range(NG):
            for kt in range(KT):
                w = gmpool.tile([128, NP], f16, name=f"wg{tg}{kt}", tag=f"wgm{tg}{kt}")
                eng = nc.sync if kt % 2 == 0 else nc.scalar
                eng.dma_start(out=w[:], in_=wg_d[tg, kt * 128:(kt + 1) * 128, :])
                wg_sb[(tg, kt)] = w
        nc.sync.dma_start(out=wmm_sb[:], in_=wmm_d[:])
        nc.scalar.dma_start(out=bias_sb[:], in_=bias_d[:])

        # ---------------- gather round helper ----------------
        # one round = 4-chain matmul over node k-tiles for channel block bp,
        # graph tg, time t, drained into the per-(tg,bp) staging tile. Rounds
        # for bp are interleaved into layer-bp's B phase (bp=0 needs only x,
        # bp=1 needs layer-1 output, ...) to keep PE fed while ACT/DVE/Pool
        # chew on activations; bp=2 runs as the tail.
        osb_map = {}

        def gather_round(tg, bp, t, copy_eng, pool):
            if (tg, bp) not in osb_map:
                osb_map[(tg, bp)] = outpool.tile([128, T, NP], f16,
                                                 name=f"osb{tg}{bp}", tag="osb")
            osb = osb_map[(tg, bp)]
            ps = pool.tile([128, NF], f32, name="psg",
                           tag="psadj" if pool is psA else "psw")
            for kt in range(KT):
                nc.tensor.matmul(
                    ps[:],
                    g[kt][:, bp, t, :, :],
                    wg_sb[(tg, kt)][:, 0:NF],
                    start=(kt == 0), stop=(kt == KT - 1),
                )
            if copy_eng == "v":
                nc.vector.tensor_copy(osb[:, t, 0:NF], ps[:])
            else:
                nc.scalar.copy(osb[:, t, 0:NF], ps[:])
            # fire output DMAs per third-T so the last exposed transfer at
            # kernel end is as small as possible
            if t % 4 == 3:
                sl = slice(t - 3, t + 1)
                for i in range(N_MIX):
                    nc.sync.dma_start(
                        out=out_d[i][tg * C + bp * DIM: tg * C + (bp + 1) * DIM, sl, :],
                        in_=osb[i * DIM:(i + 1) * DIM, sl, :],
                    )

        # ---------------- layers ----------------
        # A (adjacency), B (weight+acts+merge) and the gather rounds for
        # c-block l are fused at the t level with a skew: iteration ti runs
        # A at t=ti and, on even ti, the B work for the t-pair (ti-2, ti-1).
        # B activations/merges operate on paired psum tiles [128, 2, NP]
        # (one act instruction covers two t's -> half the op count), and the
        # B weight matmuls are interleaved with the pair's gather rounds so
        # psW banks get drain time without stalling the in-order PE queue.
        for l in range(N_LAYERS):
            ptg = []
            for tg in range(NG):
                pt = ptpool.tile([128, T, NP], f16, name=f"pt{l}{tg}", tag=f"pt{tg}")
                ptg.append(pt)
            h = hpool.tile([128, T, NP], f16, name=f"h{l}", tag="h")
            # stt below writes only cols 0:NF; zero the padded tail once so
            # the mt=3 transposes (cols 384:512) never read SBUF junk
            nc.gpsimd.memset(h[:, :, NF:NP], 0.0)

            for ti in range(T + 1):
                if ti < T:
                    for tg in range(NG):
                        ps = psA.tile([128, NF], f32, name="psadj", tag="psadj")
                        for kt in range(KT):
                            nc.tensor.matmul(
                                ps[:],
                                g[kt][:, l, ti, :, :],
                                adj_sb[(tg, kt)][:, 0:NF],
                                start=(kt == 0),
                                stop=(kt == KT - 1),
                            )
                        if (ti + tg) % 2 == 0:
                            nc.vector.tensor_copy(ptg[tg][:, ti, 0:NF], ps[:])
                        else:
                            nc.scalar.copy(ptg[tg][:, ti, 0:NF], ps[:])
                if ti >= 1:
                    t = ti - 1
                    acts = []
                    for tg in range(NG):
                        for j in range(N_MIX):
                            psc = psW.tile([128, NF], f32, name="psw", tag="psw")
                            nc.tensor.matmul(psc[:], c_wblk(l, tg, j),
                                             ptg[tg][:, t, 0:NF],
                                             start=True, stop=True)
                            a = apool.tile([128, NF], f16, name="act", tag="act")
                            if (t * 4 + tg * 2 + j) % 2 == 0:
                                nc.scalar.activation(a[:], psc[:], AF.Relu,
                                                     bias=c_bias(l, tg, j))
                            else:
                                nc.vector.scalar_tensor_tensor(
                                    a[:], psc[:], c_bias(l, tg, j), zeros[:, 0, 0:NF],
                                    op0=ALU.add, op1=ALU.max,
                                )
                            acts.append(a)
                        psd = psW.tile([128, NF], f32, name="psw2", tag="psw")
                        nc.tensor.matmul(psd[:], d_wblk(l, tg), ptg[tg][:, t, 0:NF],
                                         start=True, stop=True)
                        a = apool.tile([128, NF], f16, name="actd", tag="act")
                        nc.scalar.activation(a[:], psd[:], AF.Tanh, bias=d_bias(l, tg))
                        acts.append(a)
                    # acts = [c00, c01, d0, c10, c11, d1]
                    s0 = mpool.tile([128, NF], f16, name="s0", tag="mg")
                    nc.gpsimd.tensor_add(s0[:], acts[0][:], acts[1][:])
                    s1 = mpool.tile([128, NF], f16, name="s1", tag="mg")
                    nc.gpsimd.tensor_add(s1[:], acts[2][:], acts[5][:])
                    s2 = mpool.tile([128, NF], f16, name="s2", tag="mg")
                    nc.gpsimd.tensor_add(s2[:], acts[3][:], acts[4][:])
                    s01 = mpool.tile([128, NF], f16, name="s01", tag="mg")
                    nc.vector.tensor_add(s01[:], s0[:], s2[:])
                    nc.vector.scalar_tensor_tensor(
                        h[:, t, 0:NF], s01[:], hconst(l), s1[:],
                        op0=ALU.add, op1=ALU.add,
                    )
                    # two gather rounds for c-block l (tg 0 and 1 at t);
                    # second round drains through psW to spread bank pressure
                    gather_round(0, l, t, "v" if t % 2 == 0 else "s", psA)
                    gather_round(1, l, t, "s" if t % 2 == 0 else "v", psW)

            # transpose h back into g c-block l+1 (node-major); half-T psum
            # tiles (1 bank each) so transposes of one half overlap the
            # psum->g copy of the other
            for mt in range(KT):
                for half in range(2):
                    pst = psT.tile([128, T // 2, 128], f16, name="pstr", tag="pstr")
                    for tt in range(T // 2):
                        t = half * (T // 2) + tt
                        nc.tensor.transpose(
                            pst[:, tt, :],
                            h[:, t, mt * 128:(mt + 1) * 128],
                            ident[:],
                        )
                    nc.vector.tensor_copy(
                        g[mt][:, l + 1, half * (T // 2):(half + 1) * (T // 2), :, :],
                        pst[:].rearrange("p t (i d) -> p t i d", i=N_MIX),
                    )

        # ---------------- gather tail: c-block 2 (layer-2 output) ----------
        # tg0 fully first so its output DMAs overlap tg1's rounds, leaving
        # only tg1's final half-T transfer exposed at the end
        for tg in range(NG):
            for t in range(T):
                gather_round(tg, 2, t, "v" if t % 2 == 0 else "s",
                             psA if t % 2 == 0 else psW)

    nc.compile()
    _BUILT["nc"] = nc
    return nc


def _host_prep(x0, x1, graphs, neighbors, neighbors_weight, a_weight, B_weight,
               a_bias, B_bias):
    """Fuse weights, densify gather, build per-core input maps (fp16)."""
    f = np.float32
    h16 = np.float16
    x0 = np.asarray(x0, f)
    x1 = np.asarray(x1, f)
    graphs = np.asarray(graphs, f)
    neighbors = np.asarray(neighbors).astype(np.int64)
    neighbors_weight = np.asarray(neighbors_weight, f)
    a_weight = np.asarray(a_weight, f)
    B_weight = np.asarray(B_weight, f)
    a_bias = np.asarray(a_bias, f)
    B_bias = np.asarray(B_bias, f)

    wc = np.sum(a_weight[0] * B_weight, axis=1)        # [R, L, D, D]
    wd = np.sum(a_weight[1] * B_weight, axis=1)
    bc = np.sum(a_bias[0] * B_bias, axis=1)            # [R, L, D]
    bd = np.sum(a_bias[1] * B_bias, axis=1)

    # wmm blob: 12 blocks of [128=(j,d), 128=(i,d')], K=128 with zero halves.
    # c block (l,tg,j): rows j*64.. hold [wc(tg,0,j) | wc(tg,1,j)], rest zero.
    # d block (l,tg): rows 0:64 = [-wd(r01) | +wd(r10)], rows 64:128 = [+wd(r01) | -wd(r10)]
    wmm = np.zeros((128, 12 * 128), f)
    for l in range(N_LAYERS):
        for tg in range(NG):
            for j in range(N_MIX):
                blk = (l * NG + tg) * 2 + j
                r0 = j * 64
                wmm[r0:r0 + 64, blk * 128: blk * 128 + 64] = wc[_rel(tg, 0, j), l]
                wmm[r0:r0 + 64, blk * 128 + 64: blk * 128 + 128] = wc[_rel(tg, 1, j), l]
            blk = 8 + l * NG + tg
            wd01, wd10 = wd[_rel(tg, 0, 1), l], wd[_rel(tg, 1, 0), l]
            wmm[0:64, blk * 128: blk * 128 + 64] = -wd01
            wmm[0:64, blk * 128 + 64: blk * 128 + 128] = wd10
            wmm[64:128, blk * 128: blk * 128 + 64] = wd01
            wmm[64:128, blk * 128 + 64: blk * 128 + 128] = -wd10

    bias = np.zeros((128, 16), f)
    for l in range(N_LAYERS):
        for tg in range(NG):
            for j in range(N_MIX):
                col = (l * NG + tg) * 2 + j
                bias[0:64, col] = bc[_rel(tg, 0, j), l]
                bias[64:128, col] = bc[_rel(tg, 1, j), l]
            col = 8 + l * NG + tg
            bias[0:64, col] = bd[_rel(tg, 0, 1), l]
            bias[64:128, col] = bd[_rel(tg, 1, 0), l]
        hc = np.zeros(128, f)
        for i in range(N_MIX):
            acc = np.zeros(DIM, f)
            for tg in range(NG):
                acc += np.tanh(bd[_rel(tg, i, i), l])
            hc[i * DIM:(i + 1) * DIM] = acc
        bias[:, 12 + l] = hc

    adjp = np.zeros((NG, NP, NP), h16)
    adjp[:, :N, :N] = graphs
    wgp = np.zeros((NG, NP, NP), f)
    for tg in range(NG):
        np.add.at(
            wgp[tg],
            (neighbors[tg].reshape(-1),
             np.repeat(np.arange(N), K)),
            neighbors_weight[tg].reshape(-1),
        )
    wgp = wgp.astype(h16)
    wmm16 = wmm.astype(h16)

    in_maps = []
    for b in range(NCORES):
        xn = np.zeros((N_MIX, NP, T, DIM), h16)
        xn[0, :N] = np.transpose(x0[b], (1, 2, 0))  # [D,N,T] -> [N,T,D]
        xn[1, :N] = np.transpose(x1[b], (1, 2, 0))
        in_maps.append({
            "xn": xn, "adj": adjp, "wg": wgp, "wmm": wmm16, "bias": bias,
        })
    return in_maps


def kernel(x0, x1, graphs, neighbors, neighbors_weight, a_weight, B_weight,
           a_bias, B_bias):
    from concourse.bass_utils import run_bass_kernel_spmd

    nc = _build()
    in_maps = _host_prep(x0, x1, graphs, neighbors, neighbors_weight,
                         a_weight, B_weight, a_bias, B_bias)
    trace = bool(int(os.environ.get("KERNEL_TRACE", "0")))
    res = run_bass_kernel_spmd(nc, in_maps, list(range(NCORES)), trace=trace)
    kernel.last_result = res

    outs = []
    for i in range(N_MIX):
        # device layout [C*NG, T, NP] fp16 -> [B, C*NG, N, T] f32
        o = np.stack([
            np.asarray(res.results[b][f"out{i}"])[:, :, :N]
            .astype(np.float32).transpose(0, 2, 1)
            for b in range(NCORES)
        ])
        outs.append(o)
    return tuple(outs)


kernel.last_result = None


# revision 28
# speedup vs baseline: 1.0508x; 1.0508x over previous
"""CMRGCN Trainium2 kernel (v2, fp16 matmul path).

Sharding: data-parallel over batch B=8 across the 8 NeuronCores (core b gets
batch b). Adjacency / neighbor weights / fused relation weights replicated.

v2 changes over the f32r baseline (361 us):
  - all matmul operands fp16 (PE row rate identical, but halves SBUF/DMA and
    enables DVE 2x modes on fp16 elementwise ops)
  - elementwise spread over ACT + DVE + Pool (gpsimd) instead of ACT+DVE only;
    ReLU runs on DVE as (psum + bias) max zeros via scalar_tensor_tensor
  - gather phase: psum rotation over two pools, fp16 staging tile double-
    buffered, one big DMA per (graph, channel-block, mix) instead of per-t
    copies racing a single-buffered staging tile (the old bottleneck: PE
    stalled ~100us waiting on staging WAR + 1.5MB DMAs)
  - output layout [C, T, 512] fp16 on device; host slices/transposes back to
    [C, N, T] f32

Per-core layout (N=500 padded to NP=512):
  g   [4 x (128, cb=3, T, MIX=2, DIM)] fp16, node-major. cb 0=x, 1=h1, 2=h2.
  Adjacency matmul: lhsT = g-slice [n, (i,d)], rhs = adj [n, m] -> P^T psum
  Weight matmul:    lhsT = fused-W block [ (j,d), (i,d') ], rhs = P^T
  d-path folded as +/-W pairs (relation weights for i!=j), i==j tanh(bias)
  terms folded into a per-layer constant added at merge time.
  h [128=(i,d'), T, m] fp16 accumulated via act tmps + 5-op merge tree,
  PE-transposed back into g c-block l+1.
  Neighbor gather = dense matmul with host-densified W_g [m, n] per graph.
"""

import os
import numpy as np

B, T, N, DIM = 8, 12, 500, 64
N_MIX, N_LAYERS, N_HEADS, N_REL, NG, K = 2, 2, 4, 8, 2, 20
NP = 512          # padded node count
NF = 500          # moving-free trim: only the first N output nodes are real
KT = NP // 128    # node k-tiles
C = DIM * (N_LAYERS + 1)   # 192 channels per mix in g
NCORES = 8

_BUILT = {}


def _rel(tg, i, j):
    return (tg * N_MIX + i) * N_MIX + j


def _build():
    if "nc" in _BUILT:
        return _BUILT["nc"]

    from contextlib import ExitStack
    import concourse.bass as bass
    import concourse.tile as tile
    import concourse.mybir as mybir
    from concourse import bacc
    from concourse.masks import make_identity

    f32 = mybir.dt.float32
    f16 = mybir.dt.float16
    AF = mybir.ActivationFunctionType
    ALU = mybir.AluOpType

    nc = bacc.Bacc("TRN2", target_bir_lowering=False, debug=False)

    xn_d = nc.dram_tensor("xn", [N_MIX, NP, T, DIM], f16, kind="ExternalInput").ap()
    adj_d = nc.dram_tensor("adj", [NG, NP, NP], f16, kind="ExternalInput").ap()
    wg_d = nc.dram_tensor("wg", [NG, NP, NP], f16, kind="ExternalInput").ap()
    # W blocks [128=(j,d), 128=(i,d')], K=128 with zero halves so every matmul
    # operand sits at base partition 0 (base-64 operands crash HW)
    wmm_d = nc.dram_tensor("wmm", [128, 12 * 128], f16, kind="ExternalInput").ap()
    bias_d = nc.dram_tensor("bias", [128, 16], f32, kind="ExternalInput").ap()
    out_d = [
        nc.dram_tensor(f"out{i}", [NG * C, T, NP], f16, kind="ExternalOutput").ap()
        for i in range(N_MIX)
    ]

    with tile.TileContext(nc) as tc, ExitStack() as ctx:
        wpool = ctx.enter_context(tc.tile_pool(name="wpool", bufs=1))
        gpool = ctx.enter_context(tc.tile_pool(name="gpool", bufs=1))
        gmpool = ctx.enter_context(tc.tile_pool(name="gmpool", bufs=1))
        ptpool = ctx.enter_context(tc.tile_pool(name="ptpool", bufs=2))
        hpool = ctx.enter_context(tc.tile_pool(name="hpool", bufs=2))
        apool = ctx.enter_context(tc.tile_pool(name="apool", bufs=12))
        mpool = ctx.enter_context(tc.tile_pool(name="mpool", bufs=8))
        outpool = ctx.enter_context(tc.tile_pool(name="outpool", bufs=2))
        psA = ctx.enter_context(tc.tile_pool(name="psA", bufs=2, space="PSUM"))
        psW = ctx.enter_context(tc.tile_pool(name="psW", bufs=4, space="PSUM"))
        psT = ctx.enter_context(tc.tile_pool(name="psT", bufs=2, space="PSUM"))

        # --- constants / weights ---
        wmm_sb = wpool.tile([128, 12 * 128], f16, name="wmm_sb")
        bias_sb = wpool.tile([128, 16], f32, name="bias_sb")
        ident = wpool.tile([128, 128], f16, name="ident")
        make_identity(nc, ident[:])
        zeros = wpool.tile([128, 2, NP], f16, name="zeros")
        nc.gpsimd.memset(zeros[:], 0.0)

        def wmm_blk(idx):
            return wmm_sb[:, idx * 128:(idx + 1) * 128]

        def c_wblk(l, tg, j):
            return wmm_blk((l * NG + tg) * 2 + j)

        def d_wblk(l, tg):
            return wmm_blk(8 + l * NG + tg)

        def c_bias(l, tg, j):
            col = (l * NG + tg) * 2 + j
            return bias_sb[:, col:col + 1]

        def d_bias(l, tg):
            return bias_sb[:, 8 + l * NG + tg: 8 + l * NG + tg + 1]

        def hconst(l):
            return bias_sb[:, 12 + l:12 + l + 1]

        # --- load x into g c-block 0, adjacency + gather-weight tiles ---
        # dispatch order tuned for startup: adjacency tg0 (sync) and xn
        # (split queues) first so the first A-phase chain starts ASAP; wg
        # last (first needed ~35us in by the interleaved gather rounds)
        g = []
        for mt in range(KT):
            gt = gpool.tile([128, 3, T, N_MIX, DIM], f16, name=f"g{mt}", tag=f"g{mt}")
            g.append(gt)

        # the first A chain consumes (adj0[kt], g[kt]) in kt order, so land
        # each kt's triple (adj row-block + both x mixes) together, split
        # across the two HWDGE queues: the first matmul can start after the
        # kt=0 triple instead of after all 12 startup transfers
        adj_sb, wg_sb = {}, {}
        for kt in range(KT):
            e1 = nc.sync if kt % 2 == 0 else nc.scalar
            e2 = nc.scalar if kt % 2 == 0 else nc.sync
            a = gmpool.tile([128, NP], f16, name=f"adj0{kt}", tag=f"gm0{kt}")
            e1.dma_start(out=a[:], in_=adj_d[0, kt * 128:(kt + 1) * 128, :])
            adj_sb[(0, kt)] = a
            e2.dma_start(
                out=g[kt][:, 0, :, 0, :],
                in_=xn_d[0, kt * 128:(kt + 1) * 128, :, :],
            )
            e1.dma_start(
                out=g[kt][:, 0, :, 1, :],
                in_=xn_d[1, kt * 128:(kt + 1) * 128, :, :],
            )
        for kt in range(KT):
            a = gmpool.tile([128, NP], f16, name=f"adj1{kt}", tag=f"gm1{kt}")
            eng = nc.scalar if kt % 2 == 0 else nc.sync
            eng.dma_start(out=a[:], in_=adj_d[1, kt * 128:(kt + 1) * 128, :])
            adj_sb[(1, kt)] = a
        for tg in range(NG):
            for kt in range(KT):
                w = gmpool.tile([128, NP], f16, name=f"wg{tg}{kt}", tag=f"wgm{tg}{kt}")
                eng = nc.sync if kt % 2 == 0 else nc.scalar
                eng.dma_start(out=w[:], in_=wg_d[tg, kt * 128:(kt + 1) * 128, :])
                wg_sb[(tg, kt)] = w
        nc.sync.dma_start(out=wmm_sb[:], in_=wmm_d[:])
        nc.scalar.dma_start(out=bias_sb[:], in_=bias_d[:])

        # ---------------- gather round helper ----------------
        # one round = 4-chain matmul over node k-tiles for channel block bp,
        # graph tg, time t, drained into the per-(tg,bp) staging tile. Rounds
        # for bp are interleaved into layer-bp's B phase (bp=0 needs only x,
        # bp=1 needs layer-1 output, ...) to keep PE fed while ACT/DVE/Pool
        # chew on activations; bp=2 runs as the tail.
        osb_map = {}

        def gather_round(tg, bp, t, copy_eng, pool):
            if (tg, bp) not in osb_map:
                osb_map[(tg, bp)] = outpool.tile([128, T, NP], f16,
                                                 name=f"osb{tg}{bp}", tag="osb")
            osb = osb_map[(tg, bp)]
            ps = pool.tile([128, NF], f32, name="psg",
                           tag="psadj" if pool is psA else "psw")
            for kt in range(KT):
                nc.tensor.matmul(
                    ps[:],
                    g[kt][:, bp, t, :, :],
                    wg_sb[(tg, kt)][:, 0:NF],
                    start=(kt == 0), stop=(kt == KT - 1),
                )
            if copy_eng == "v":
                nc.vector.tensor_copy(osb[:, t, 0:NF], ps[:])
            else:
                nc.scalar.copy(osb[:, t, 0:NF], ps[:])
            # fire output DMAs per half-T so the last transfer overlaps compute
            if t == T // 2 - 1 or t == T - 1:
                sl = slice(0, T // 2) if t == T // 2 - 1 else slice(T // 2, T)
                for i in range(N_MIX):
                    nc.sync.dma_start(
                        out=out_d[i][tg * C + bp * DIM: tg * C + (bp + 1) * DIM, sl, :],
                        in_=osb[i * DIM:(i + 1) * DIM, sl, :],
                    )

        # ---------------- layers ----------------
        # A (adjacency), B (weight+acts+merge) and the gather rounds for
        # c-block l are fused at the t level with a skew: iteration ti runs
        # A at t=ti and, on even ti, the B work for the t-pair (ti-2, ti-1).
        # B activations/merges operate on paired psum tiles [128, 2, NP]
        # (one act instruction covers two t's -> half the op count), and the
        # B weight matmuls are interleaved with the pair's gather rounds so
        # psW banks get drain time without stalling the in-order PE queue.
        for l in range(N_LAYERS):
            ptg = []
            for tg in range(NG):
                pt = ptpool.tile([128, T, NP], f16, name=f"pt{l}{tg}", tag=f"pt{tg}")
                ptg.append(pt)
            h = hpool.tile([128, T, NP], f16, name=f"h{l}", tag="h")
            # stt below writes only cols 0:NF; zero the padded tail once so
            # the mt=3 transposes (cols 384:512) never read SBUF junk
            nc.gpsimd.memset(h[:, :, NF:NP], 0.0)

            for ti in range(T + 1):
                if ti < T:
                    for tg in range(NG):
                        ps = psA.tile([128, NF], f32, name="psadj", tag="psadj")
                        for kt in range(KT):
                            nc.tensor.matmul(
                                ps[:],
                                g[kt][:, l, ti, :, :],
                                adj_sb[(tg, kt)][:, 0:NF],
                                start=(kt == 0),
                                stop=(kt == KT - 1),
                            )
                        if (ti + tg) % 2 == 0:
                            nc.vector.tensor_copy(ptg[tg][:, ti, 0:NF], ps[:])
                        else:
                            nc.scalar.copy(ptg[tg][:, ti, 0:NF], ps[:])
                if ti >= 1:
                    t = ti - 1
                    acts = []
                    for tg in range(NG):
                        for j in range(N_MIX):
                            psc = psW.tile([128, NF], f32, name="psw", tag="psw")
                            nc.tensor.matmul(psc[:], c_wblk(l, tg, j),
                                             ptg[tg][:, t, 0:NF],
                                             start=True, stop=True)
                            a = apool.tile([128, NF], f16, name="act", tag="act")
                            if (t * 4 + tg * 2 + j) % 2 == 0:
                                nc.scalar.activation(a[:], psc[:], AF.Relu,
                                                     bias=c_bias(l, tg, j))
                            else:
                                nc.vector.scalar_tensor_tensor(
                                    a[:], psc[:], c_bias(l, tg, j), zeros[:, 0, 0:NF],
                                    op0=ALU.add, op1=ALU.max,
                                )
                            acts.append(a)
                        psd = psW.tile([128, NF], f32, name="psw2", tag="psw")
                        nc.tensor.matmul(psd[:], d_wblk(l, tg), ptg[tg][:, t, 0:NF],
                                         start=True, stop=True)
                        a = apool.tile([128, NF], f16, name="actd", tag="act")
                        nc.scalar.activation(a[:], psd[:], AF.Tanh, bias=d_bias(l, tg))
                        acts.append(a)
                    # acts = [c00, c01, d0, c10, c11, d1]
                    s0 = mpool.tile([128, NF], f16, name="s0", tag="mg")
                    nc.gpsimd.tensor_add(s0[:], acts[0][:], acts[1][:])
                    s1 = mpool.tile([128, NF], f16, name="s1", tag="mg")
                    nc.gpsimd.tensor_add(s1[:], acts[2][:], acts[5][:])
                    s2 = mpool.tile([128, NF], f16, name="s2", tag="mg")
                    nc.gpsimd.tensor_add(s2[:], acts[3][:], acts[4][:])
                    s01 = mpool.tile([128, NF], f16, name="s01", tag="mg")
                    nc.vector.tensor_add(s01[:], s0[:], s2[:])
                    nc.vector.scalar_tensor_tensor(
                        h[:, t, 0:NF], s01[:], hconst(l), s1[:],
                        op0=ALU.add, op1=ALU.add,
                    )
                    # two gather rounds for c-block l (tg 0 and 1 at t)
                    gather_round(0, l, t, "v" if t % 2 == 0 else "s", psA)
                    gather_round(1, l, t, "s" if t % 2 == 0 else "v", psA)

            # transpose h back into g c-block l+1 (node-major); half-T psum
            # tiles (1 bank each) so transposes of one half overlap the
            # psum->g copy of the other
            for mt in range(KT):
                for half in range(2):
                    pst = psT.tile([128, T // 2, 128], f16, name="pstr", tag="pstr")
                    for tt in range(T // 2):
                        t = half * (T // 2) + tt
                        nc.tensor.transpose(
                            pst[:, tt, :],
                            h[:, t, mt * 128:(mt + 1) * 128],
                            ident[:],
                        )
                    nc.vector.tensor_copy(
                        g[mt][:, l + 1, half * (T // 2):(half + 1) * (T // 2), :, :],
                        pst[:].rearrange("p t (i d) -> p t i d", i=N_MIX),
                    )

        # ---------------- gather tail: c-block 2 (layer-2 output) ----------
        # tg0 fully first so its output DMAs overlap tg1's rounds, leaving
        # only tg1's final half-T transfer exposed at the end
        for tg in range(NG):
            for t in range(T):
                gather_round(tg, 2, t, "v" if t % 2 == 0 else "s",
                             psA if t % 2 == 0 else psW)

    nc.compile()
    _BUILT["nc"] = nc
    return nc


def _host_prep(x0, x1, graphs, neighbors, neighbors_weight, a_weight, B_weight,
               a_bias, B_bias):
    """Fuse weights, densify gather, build per-core input maps (fp16)."""
    f = np.float32
    h16 = np.float16
    x0 = np.asarray(x0, f)
    x1 = np.asarray(x1, f)
    graphs = np.asarray(graphs, f)
    neighbors = np.asarray(neighbors).astype(np.int64)
    neighbors_weight = np.asarray(neighbors_weight, f)
    a_weight = np.asarray(a_weight, f)
    B_weight = np.asarray(B_weight, f)
    a_bias = np.asarray(a_bias, f)
    B_bias = np.asarray(B_bias, f)

    wc = np.sum(a_weight[0] * B_weight, axis=1)        # [R, L, D, D]
    wd = np.sum(a_weight[1] * B_weight, axis=1)
    bc = np.sum(a_bias[0] * B_bias, axis=1)            # [R, L, D]
    bd = np.sum(a_bias[1] * B_bias, axis=1)

    # wmm blob: 12 blocks of [128=(j,d), 128=(i,d')], K=128 with zero halves.
    # c block (l,tg,j): rows j*64.. hold [wc(tg,0,j) | wc(tg,1,j)], rest zero.
    # d block (l,tg): rows 0:64 = [-wd(r01) | +wd(r10)], rows 64:128 = [+wd(r01) | -wd(r10)]
    wmm = np.zeros((128, 12 * 128), f)
    for l in range(N_LAYERS):
        for tg in range(NG):
            for j in range(N_MIX):
                blk = (l * NG + tg) * 2 + j
                r0 = j * 64
                wmm[r0:r0 + 64, blk * 128: blk * 128 + 64] = wc[_rel(tg, 0, j), l]
                wmm[r0:r0 + 64, blk * 128 + 64: blk * 128 + 128] = wc[_rel(tg, 1, j), l]
            blk = 8 + l * NG + tg
            wd01, wd10 = wd[_rel(tg, 0, 1), l], wd[_rel(tg, 1, 0), l]
            wmm[0:64, blk * 128: blk * 128 + 64] = -wd01
            wmm[0:64, blk * 128 + 64: blk * 128 + 128] = wd10
            wmm[64:128, blk * 128: blk * 128 + 64] = wd01
            wmm[64:128, blk * 128 + 64: blk * 128 + 128] = -wd10

    bias = np.zeros((128, 16), f)
    for l in range(N_LAYERS):
        for tg in range(NG):
            for j in range(N_MIX):
                col = (l * NG + tg) * 2 + j
                bias[0:64, col] = bc[_rel(tg, 0, j), l]
                bias[64:128, col] = bc[_rel(tg, 1, j), l]
            col = 8 + l * NG + tg
            bias[0:64, col] = bd[_rel(tg, 0, 1), l]
            bias[64:128, col] = bd[_rel(tg, 1, 0), l]
        hc = np.zeros(128, f)
        for i in range(N_MIX):
            acc = np.zeros(DIM, f)
            for tg in range(NG):
                acc += np.tanh(bd[_rel(tg, i, i), l])
            hc[i * DIM:(i + 1) * DIM] = acc
        bias[:, 12 + l] = hc

    adjp = np.zeros((NG, NP, NP), h16)
    adjp[:, :N, :N] = graphs
    wgp = np.zeros((NG, NP, NP), f)
    for tg in range(NG):
        np.add.at(
            wgp[tg],
            (neighbors[tg].reshape(-1),
             np.repeat(np.arange(N), K)),
            neighbors_weight[tg].reshape(-1),
        )
    wgp = wgp.astype(h16)
    wmm16 = wmm.astype(h16)

    in_maps = []
    for b in range(NCORES):
        xn = np.zeros((N_MIX, NP, T, DIM), h16)
        xn[0, :N] = np.transpose(x0[b], (1, 2, 0))  # [D,N,T] -> [N,T,D]
        xn[1, :N] = np.transpose(x1[b], (1, 2, 0))
        in_maps.append({
            "xn": xn, "adj": adjp, "wg": wgp, "wmm": wmm16, "bias": bias,
        })
    return in_maps


def kernel(x0, x1, graphs, neighbors, neighbors_weight, a_weight, B_weight,
           a_bias, B_bias):
    from concourse.bass_utils import run_bass_kernel_spmd

    nc = _build()
    in_maps = _host_prep(x0, x1, graphs, neighbors, neighbors_weight,
                         a_weight, B_weight, a_bias, B_bias)
    trace = bool(int(os.environ.get("KERNEL_TRACE", "0")))
    res = run_bass_kernel_spmd(nc, in_maps, list(range(NCORES)), trace=trace)
    kernel.last_result = res

    outs = []
    for i in range(N_MIX):
        # device layout [C*NG, T, NP] fp16 -> [B, C*NG, N, T] f32
        o = np.stack([
            np.asarray(res.results[b][f"out{i}"])[:, :, :N]
            .astype(np.float32).transpose(0, 2, 1)
            for b in range(NCORES)
        ])
        outs.append(o)
    return tuple(outs)


kernel.last_result = None
